# revision 4
# baseline (speedup 1.0000x reference)
"""Trainium2 Bass kernel: decode-step attention with static KV cache (GQA).

Problem shapes (hardcoded):
  x        [16, 1, 4096]      activations (B=16, QLEN=1, DIM=4096)
  cache_k  [16, 8192, 8, 128] K cache (PREFIX=8192, HKV=8, HD=128)
  cache_v  [16, 8192, 8, 128]
  wq       [4096, 4096]  (H*HD, DIM), H=32
  wk/wv    [1024, 4096]
  wo       [4096, 4096]  (DIM, H*HD)
  out      [16, 1, 4096]

Sharding: tensor-parallel over the kv-head axis. Core c owns kv head c and
q heads 4c..4c+3; weights are column/row-sliced per core, the KV slice is
extracted per core on the host (K transposed to [d, t] with an interleaved
column order, see below). Each core computes a partial [16, 4096] output;
the host sums the 8 partials.

Dtype strategy (the problem is HBM-bandwidth bound, so bytes == time):
  - All device compute dtypes are f16 except V, which is stored in HBM as
    fp8 E3M4 (4-bit mantissa; V ~ N(0,1), |V|max ~ 5.8 << 15.5 range).
    The PV matmul streams fp8 V against f16 P (mixed operand dtypes are
    legal on TRN2; both upcast to FP22 in the PE).
  - Casting f32 -> f16/f8 happens on the HOST, so HBM only ever stores and
    the DMA engines only ever move the narrow types: per-core traffic drops
    from 148 MB (f32) to 58 MB.
  - All HBM tensors are pre-packed on the host into the exact SBUF tile
    layout, so every load is a single fully-contiguous DMA on one HWDGE
    ring, queued in consumption order.

t-ordering: V loads as [128, (n d1)] with t = 64*p + n and d1 = d plus a
trailing ones column (d1 = 129).  The ones column makes the PV matmul
accumulate the softmax denominator into op[:, 128] for free.  The host
permutes K's columns to the same t order, so score tiles and V tiles
agree on partition<->t mapping.

Per-core dataflow (software-pipelined on the PE so it never stalls):
  phase 0: q/k_new/v_new projections (f16 PE), transposes to get
           qT[d,(h,b)], kT_new[d,b], v_new rows (|1) in f16.
  loop b:  scores(b):  64+1 f16 matmuls -> PSUM [t-tile, h];
                       exp (ACT, scale=1/sqrt(128)) -> P f16
           tt(b-2):    PE transpose of the scaled attn rows into AT
           pv(b-1):    64+1 matmuls accumulate [h, d+1] f32 PSUM;
                       DVE: rc = 1/op[:,128], ao = op[:,:128]*rc (async)
  phase 2: out = AT-chunks.T @ woT (f16, resident wo), DMA out.
"""

import os
import sys

_REPO = "/opt/trn_rl_repo"
if _REPO not in sys.path:
    sys.path.insert(0, _REPO)

import numpy as np
import ml_dtypes

import concourse.bacc as bacc
import concourse.mybir as mybir
import concourse.tile as tile
from concourse.bass_utils import run_bass_kernel_spmd
from concourse.masks import make_identity

B = 16          # batch
T = 8192        # prefix length in cache
NT = T // 128   # 64 K/V tiles per batch
HD = 128        # head dim
HD1 = HD + 1    # head dim + denominator ones column
HQ = 4          # q heads per core
DIM = 4096
NDT = DIM // 128  # 32 contraction tiles for the projections
NCORES = 8
F32 = mybir.dt.float32
F16 = mybir.dt.float16
F8E3 = mybir.dt.float8e3
SCALE = 1.0 / float(np.sqrt(128.0))
SW = 4 * NT + 4   # score tile width: 64 cache tiles + new token, 4 heads each
NWC = 8           # dt-tiles per wq chunk

# V storage dtype: fp8 e3m4 (1 byte) by default; "f16" for the safe config.
_VDT_NAME = os.environ.get("KVD", "f8e3")
VDT = F8E3 if _VDT_NAME == "f8e3" else F16
VDT_NP = ml_dtypes.float8_e3m4 if _VDT_NAME == "f8e3" else np.float16

Exp = mybir.ActivationFunctionType.Exp
Mult = mybir.AluOpType.mult


def _build_nc():
    nc = bacc.Bacc("TRN2", target_bir_lowering=False, debug=False)

    # All inputs pre-packed on host into SBUF tile layout (partition-major).
    xs = nc.dram_tensor("xs", [128, NDT * B], F16, kind="ExternalInput")
    wqp = nc.dram_tensor("wqp", [NDT // NWC, 128, NWC * HQ * HD], F16,
                         kind="ExternalInput")
    wkp = nc.dram_tensor("wkp", [128, NDT * HD], F16, kind="ExternalInput")
    wvp = nc.dram_tensor("wvp", [128, NDT * HD], F16, kind="ExternalInput")
    wop = nc.dram_tensor("wop", [128, HQ * DIM], F16, kind="ExternalInput")
    kT = nc.dram_tensor("kT", [B, HD, T], F16, kind="ExternalInput")
    v = nc.dram_tensor("v", [B, 128, NT * HD1], VDT, kind="ExternalInput")
    out = nc.dram_tensor("out", [B, DIM], F32, kind="ExternalOutput")

    with tile.TileContext(nc) as tc:
        _emit(nc, tc, xs, wqp, wkp, wvp, wop, kT, v, out)
    nc.compile()
    return nc


def _emit(nc, tc, xs, wqp, wkp, wvp, wop, kT, v, out):
    from contextlib import ExitStack

    with ExitStack() as ctx:
        const = ctx.enter_context(tc.tile_pool(name="const", bufs=1))
        wpool = ctx.enter_context(tc.tile_pool(name="weights", bufs=3))
        wopool = ctx.enter_context(tc.tile_pool(name="wopool", bufs=2))
        kpool = ctx.enter_context(tc.tile_pool(name="kpool", bufs=3))
        vpool = ctx.enter_context(tc.tile_pool(name="vpool", bufs=3))

        ident = const.tile([16, 16], F32, tag="ident")

        # x^T in f16: [128, (dt, b)] — single contiguous load
        xs_h = const.tile([128, NDT * B], F16, tag="xs_h")
        nc.sync.dma_start(xs_h[:], xs[:])

        QT = const.tile([128, HQ * B], F32, tag="QT")       # [d, (h,b)] fp32
        QTh = const.tile([128, HQ * B], F16, tag="QTh")     # fp16 copy
        KTnh = const.tile([128, B], F16, tag="KTnh")        # new-token K^T f16
        vrowh = const.tile([1, B * HD1], F16, tag="vrowh")  # new-token V|1 f16
        AT = const.tile([128, HQ * B], F16, tag="AT")       # attn out^T f16
        wo_h = const.tile([128, HQ * DIM], F16, tag="wo_h")  # resident f16 wo
        wk_h = const.tile([128, NDT * HD], F16, tag="wk_h")
        wv_h = const.tile([128, NDT * HD], F16, tag="wv_h")
        q_s = const.tile([B, HQ * HD], F32, tag="q_s")
        kn_s = const.tile([B, HD], F32, tag="kn_s")
        vn_s = const.tile([B, HD], F32, tag="vn_s")

        make_identity(nc, ident[:])
        nc.vector.memset(vrowh[:], 1.0)

        kt_t, vb_t, pt_t, op_t, ao_t = {}, {}, {}, {}, {}

        def load(b):
            # Contiguous loads, consumption order, single HWDGE ring.
            # (K columns pre-permuted on host to the t = 64*p + n order
            # that V's layout produces.)
            ktb = kpool.tile([128, T], F16, tag="ktb")
            nc.sync.dma_start(ktb[:], kT[b])
            kt_t[b] = ktb
            vb = vpool.tile([128, NT * HD1], VDT, tag="vb")
            nc.sync.dma_start(vb[:], v[b])
            vb_t[b] = vb

        # ---------------- phase 0: projections (f16 PE) ----------------
        with tc.tile_pool(name="psum0", bufs=1, space="PSUM") as pp0:
            qp = pp0.tile([B, HQ * HD], F32, tag="qp")
            knp = pp0.tile([B, HD], F32, tag="knp")
            vnp = pp0.tile([B, HD], F32, tag="vnp")

            for c in range(NDT // NWC):
                wq_h = wpool.tile([128, NWC * HQ * HD], F16, tag="wq_h")
                nc.sync.dma_start(wq_h[:], wqp[c])
                for t in range(NWC):
                    dt = c * NWC + t
                    nc.tensor.matmul(
                        qp[:], xs_h[:, dt * B:(dt + 1) * B],
                        wq_h[:, t * HQ * HD:(t + 1) * HQ * HD],
                        start=(dt == 0), stop=(dt == NDT - 1),
                    )

            # DMA ring: K0/V0 ahead of wk/wv so scores(0) starts earliest
            load(0)
            nc.sync.dma_start(wk_h[:], wkp[:])
            nc.sync.dma_start(wv_h[:], wvp[:])
            load(1)
            load(2)

            for dt in range(NDT):
                nc.tensor.matmul(
                    knp[:], xs_h[:, dt * B:(dt + 1) * B],
                    wk_h[:, dt * HD:(dt + 1) * HD],
                    start=(dt == 0), stop=(dt == NDT - 1),
                )
            for dt in range(NDT):
                nc.tensor.matmul(
                    vnp[:], xs_h[:, dt * B:(dt + 1) * B],
                    wv_h[:, dt * HD:(dt + 1) * HD],
                    start=(dt == 0), stop=(dt == NDT - 1),
                )

            nc.vector.tensor_copy(q_s[:], qp[:])
            nc.vector.tensor_copy(kn_s[:], knp[:])
            nc.vector.tensor_copy(vn_s[:], vnp[:])

            # v_new rows (f16) flattened onto partition 0 (SWDGE cast DMA);
            # column 128 of each b stays at the memset 1.0 (denominator).
            nc.gpsimd.dma_start(
                vrowh[:].rearrange("p (b c) -> p b c", c=HD1)[0:1, :, 0:HD],
                vn_s[:],
            )

            # transposes: q [16,512] -> QT [128, (h,b)]; k_new -> KTn (f16)
            for h in range(HQ):
                tp = pp0.tile([128, B], F32, tag="tp", bufs=2)
                nc.tensor.transpose(
                    tp[:], q_s[:, h * HD:(h + 1) * HD], ident[:]
                )
                nc.vector.tensor_copy(QT[:, h * B:(h + 1) * B], tp[:])
            tpk = pp0.tile([128, B], F32, tag="tp", bufs=2)
            nc.tensor.transpose(tpk[:], kn_s[:], ident[:])
            nc.vector.tensor_copy(KTnh[:], tpk[:])
            nc.vector.tensor_copy(QTh[:], QT[:])

        # ---------------- phase 1: attention over the cache ----------------
        # Software-pipelined: burst b = [scores(b) | tt(b-2) | pv(b-1)] has
        # no PE-blocking dependencies; DVE scaling for pv(b-1) runs during
        # burst b+1, so the PE transpose in burst b+1... (lag-2 for safety).
        QTh3 = QTh[:].rearrange("p (h b) -> p b h", b=B)   # [128, b, 4]
        vrowh3 = vrowh[:].rearrange("p (b c) -> p b c", c=HD1)
        AT3 = AT[:].rearrange("p (h b) -> p b h", b=B)

        with (
            tc.tile_pool(name="ptpool", bufs=2) as ptpool,
            tc.tile_pool(name="small", bufs=3) as small,
            tc.tile_pool(name="stpsum", bufs=2, space="PSUM") as stpsum,
            tc.tile_pool(name="opsum", bufs=2, space="PSUM") as opsum,
            tc.tile_pool(name="ttpsum", bufs=2, space="PSUM") as ttpsum,
        ):
            def scores(b):
                # scores^T tiles: [t'(128), h(4)] per cache tile + new token
                stp = stpsum.tile([128, SW], F32, tag="stp")
                qb = QTh3[:, b, :]
                ktb = kt_t.pop(b)
                for n in range(NT):
                    nc.tensor.matmul(
                        stp[:, 4 * n:4 * n + 4],
                        ktb[:, 128 * n:128 * (n + 1)],
                        qb,
                        start=True, stop=True,
                    )
                nc.tensor.matmul(
                    stp[0:1, 4 * NT:SW], KTnh[:, b:b + 1], qb,
                    start=True, stop=True,
                )
                pt = ptpool.tile([128, SW], F16, tag="pt")
                nc.scalar.activation(pt[:, 0:4 * NT], stp[:, 0:4 * NT], Exp,
                                     scale=SCALE)
                nc.scalar.activation(
                    pt[0:1, 4 * NT:SW], stp[0:1, 4 * NT:SW], Exp, scale=SCALE,
                )
                pt_t[b] = pt

            def pv(b):
                # op [h(4), 129]: col 128 accumulates the denominator
                pt = pt_t.pop(b)
                vb = vb_t.pop(b)
                op = opsum.tile([HQ, HD1], F32, tag="op")
                for n in range(NT):
                    nc.tensor.matmul(
                        op[:],
                        pt[:, 4 * n:4 * n + 4],
                        vb[:, HD1 * n:HD1 * (n + 1)],
                        start=(n == 0), stop=False,
                    )
                nc.tensor.matmul(
                    op[:], pt[0:1, 4 * NT:SW], vrowh3[0:1, b, :],
                    start=False, stop=True,
                )
                op_t[b] = op
                # async DVE scaling; runs while the next burst computes
                rc = small.tile([HQ, 1], F32, tag="rc")
                nc.vector.reciprocal(rc[:], op[:, HD:HD1])
                ao = small.tile([HQ, HD], F32, tag="ao")
                nc.vector.tensor_scalar(
                    out=ao[:], in0=op[:, 0:HD], scalar1=rc[:], scalar2=None,
                    op0=Mult,
                )
                ao_t[b] = ao

            def ttst(b):
                # PE transpose of the (already scaled) attn rows; by lag 2
                # the DVE results are long since ready -> no PE stall.
                ao = ao_t.pop(b)
                op_t.pop(b)
                tt = ttpsum.tile([128, HQ], F32, tag="tt")
                nc.tensor.transpose(tt[:], ao[:], ident[0:HQ, 0:HQ])
                nc.vector.tensor_copy(AT3[:, b, :], tt[:])

            for b in range(B):
                scores(b)
                if b >= 2:
                    ttst(b - 2)
                if b >= 1:
                    pv(b - 1)
                if b + 3 < B:
                    load(b + 3)
                if b == 8:
                    # wo f16, one contiguous load queued mid-ring so it is
                    # resident well before phase 2
                    nc.sync.dma_start(wo_h[:], wop[:])
            pv(B - 1)
            ttst(B - 2)
            ttst(B - 1)

        # ---------------- phase 2: output projection (f16, resident wo) ----
        with (
            tc.tile_pool(name="wopsum", bufs=2, space="PSUM") as wopsum,
        ):
            for q in range(4):                      # 1024-col output blocks
                wop_t = wopsum.tile([B, 1024], F32, tag="wop")
                for c in range(HQ):
                    for ns in range(2):
                        nc.tensor.matmul(
                            wop_t[:, 512 * ns:512 * (ns + 1)],
                            AT[:, B * c:B * (c + 1)],
                            wo_h[:, c * DIM + 1024 * q + 512 * ns:
                                 c * DIM + 1024 * q + 512 * (ns + 1)],
                            start=(c == 0), stop=(c == HQ - 1),
                        )
                wos = wopool.tile([B, 1024], F32, tag="wos")
                nc.vector.tensor_copy(wos[:], wop_t[:])
                nc.sync.dma_start(out[:, 1024 * q:1024 * (q + 1)], wos[:])

_NC = None


def _get_nc():
    global _NC
    if _NC is None:
        _NC = _build_nc()
    return _NC


def make_in_maps(inputs):
    x = np.asarray(inputs["x"], dtype=np.float32)
    ck = np.asarray(inputs["cache_k"], dtype=np.float32)
    cv = np.asarray(inputs["cache_v"], dtype=np.float32)
    wq = np.asarray(inputs["wq"], dtype=np.float32)
    wk = np.asarray(inputs["wk"], dtype=np.float32)
    wv = np.asarray(inputs["wv"], dtype=np.float32)
    wo = np.asarray(inputs["wo"], dtype=np.float32)

    # xs: x^T [DIM, B] -> [128, (dt b)] f16
    xT = x.reshape(B, DIM).T.astype(np.float16)
    xs = np.ascontiguousarray(
        xT.reshape(NDT, 128, B).transpose(1, 0, 2).reshape(128, NDT * B)
    )
    wqT = wq.T.astype(np.float16)    # [DIM, H*HD]
    wkT = wk.T.astype(np.float16)    # [DIM, HKV*HD]
    wvT = wv.T.astype(np.float16)

    in_maps = []
    for c in range(NCORES):
        hq0 = HQ * HD * c
        # wq slice packed to chunk layout [4][128][NWC*512]
        wqc = wqT[:, hq0:hq0 + HQ * HD]          # [4096, 512]
        wqp = np.ascontiguousarray(
            wqc.reshape(NDT // NWC, NWC, 128, HQ * HD)
            .transpose(0, 2, 1, 3)
            .reshape(NDT // NWC, 128, NWC * HQ * HD)
        )
        wkc = wkT[:, HD * c:HD * (c + 1)]        # [4096, 128]
        wkp = np.ascontiguousarray(
            wkc.reshape(NDT, 128, HD).transpose(1, 0, 2).reshape(128, NDT * HD)
        )
        wvc = wvT[:, HD * c:HD * (c + 1)]
        wvp = np.ascontiguousarray(
            wvc.reshape(NDT, 128, HD).transpose(1, 0, 2).reshape(128, NDT * HD)
        )
        # wo slice [512, 4096] -> [128, (c 4096)]
        woc = wo[:, hq0:hq0 + HQ * HD].T.astype(np.float16)   # [512, 4096]
        wop = np.ascontiguousarray(
            woc.reshape(HQ, 128, DIM).transpose(1, 0, 2).reshape(128, HQ * DIM)
        )
        # K^T with columns permuted to the t = 64*p + n interleaved order
        # (matches V's layout partition mapping).
        kTc = ck[:, :, c, :].transpose(0, 2, 1).astype(np.float16)  # [B,128d,8192t]
        kTc = np.ascontiguousarray(
            kTc.reshape(B, HD, 128, NT).transpose(0, 1, 3, 2).reshape(B, HD, T)
        )
        # V layout [B, 128, (n d1)] with t = 64p + n and a trailing ones
        # column per tile (softmax denominator accumulator).
        vz = np.ones((B, 128, NT, HD1), dtype=VDT_NP)
        vz[:, :, :, 0:HD] = (
            cv[:, :, c, :].reshape(B, 128, NT, HD).astype(VDT_NP)
        )
        in_maps.append({
            "xs": xs,
            "wqp": wqp,
            "wkp": wkp,
            "wvp": wvp,
            "wop": wop,
            "kT": kTc,
            "v": vz.reshape(B, 128, NT * HD1),
        })
    return in_maps


def run(in_maps, trace=False):
    nc = _get_nc()
    return run_bass_kernel_spmd(nc, in_maps, list(range(NCORES)), trace=trace)


def kernel(**inputs):
    res = run(make_in_maps(inputs)).results
    acc = np.zeros((B, DIM), dtype=np.float64)
    for r in res:
        acc += r["out"]
    return acc.astype(np.float32).reshape(B, 1, DIM)


# revision 15
# speedup vs baseline: 1.2292x; 1.2292x over previous
"""Trainium2 Bass kernel: decode-step attention with static KV cache (GQA).

Problem shapes (hardcoded):
  x        [16, 1, 4096]      activations (B=16, QLEN=1, DIM=4096)
  cache_k  [16, 8192, 8, 128] K cache (PREFIX=8192, HKV=8, HD=128)
  cache_v  [16, 8192, 8, 128]
  wq       [4096, 4096]  (H*HD, DIM), H=32
  wk/wv    [1024, 4096]
  wo       [4096, 4096]  (DIM, H*HD)
  out      [16, 1, 4096]

Sharding: tensor-parallel over the kv-head axis. Core c owns kv head c and
q heads 4c..4c+3; weights are column/row-sliced per core, the KV slice is
extracted per core on the host (K transposed to [d, t] with an interleaved
column order, see below). Each core computes a partial [16, 4096] output;
the host sums the 8 partials.

Dtype strategy (the problem is HBM-bandwidth bound, so bytes == time):
  - All device compute dtypes are f16 except V, which is stored in HBM as
    fp8 E3M4 (4-bit mantissa; V ~ N(0,1), |V|max ~ 5.8 << 15.5 range).
    The PV matmul streams fp8 V against f16 P (mixed operand dtypes are
    legal on TRN2; both upcast to FP22 in the PE).
  - Casting f32 -> f16/f8 happens on the HOST, so HBM only ever stores and
    the DMA engines only ever move the narrow types: per-core traffic drops
    from 148 MB (f32) to 58 MB.
  - All HBM tensors are pre-packed on the host into the exact SBUF tile
    layout, so every load is a single fully-contiguous DMA on one HWDGE
    ring, queued in consumption order.

t-ordering: V loads as [128, (n d)] with t = 64*p + n.  The host permutes
K's columns to the same t order, so score tiles and V tiles agree on
partition<->t mapping.

PV matmul orientation: V tiles are the STATIONARY operand (128-column
fp8 weights get the fast-weight-load path) and the 4-column P tile is
the moving operand; the output lands as [d(128), h(4)] -- already
transposed for the wo projection, so no PE transpose is needed.  The
per-head 1/denominator scale is deferred: reciprocals of all batches
collect into one [1, 64] row, a single ones-matmul broadcasts it to
[128, 64] PSUM, and one DVE multiply rescales AT before phase 2.

Per-core dataflow (software-pipelined on the PE so it never stalls):
  phase 0: q/k_new/v_new projections (f16 PE), transposes to get
           qT[d,(h,b)], kT_new[d,b], v_new rows in f16.
  loop b:  scores(b):  64+1 f16 matmuls -> PSUM [t-tile, h];
                       exp (ACT, scale=1/sqrt(128)) -> P f16
           pv(b-1):    64+1 matmuls accumulate [d, h] f32 PSUM;
                       ones-matmul denominators; DVE: reduce, recip,
                       AT copy (all async, off the PE critical path)
  phase 2: broadcast-scale AT, then out = AT.T @ woT (f16), DMA out.
"""

import os
import sys

_REPO = "/opt/trn_rl_repo"
if _REPO not in sys.path:
    sys.path.insert(0, _REPO)

import numpy as np
import ml_dtypes

import concourse.bacc as bacc
import concourse.mybir as mybir
import concourse.tile as tile
from concourse.bass_utils import run_bass_kernel_spmd
from concourse.masks import make_identity

B = 16          # batch
T = 8192        # prefix length in cache
NT = T // 128   # 64 K/V tiles per batch
HD = 128        # head dim
HQ = 4          # q heads per core
DIM = 4096
NDT = DIM // 128  # 32 contraction tiles for the projections
NCORES = 8
F32 = mybir.dt.float32
F16 = mybir.dt.float16
F8E3 = mybir.dt.float8e3
SCALE = 1.0 / float(np.sqrt(128.0))
SW = 4 * NT + 4   # score tile width: 64 cache tiles + new token, 4 heads each
NWC = 8           # dt-tiles per wq chunk

# V storage dtype: fp8 e3m4 (1 byte) by default; "f16" for the safe config.
_VDT_NAME = os.environ.get("KVD", "f8e3")
VDT = F8E3 if _VDT_NAME == "f8e3" else F16
VDT_NP = ml_dtypes.float8_e3m4 if _VDT_NAME == "f8e3" else np.float16

Exp = mybir.ActivationFunctionType.Exp
Mult = mybir.AluOpType.mult


def _build_nc():
    nc = bacc.Bacc("TRN2", target_bir_lowering=False, debug=False)

    # All inputs pre-packed on host into SBUF tile layout (partition-major).
    xs = nc.dram_tensor("xs", [128, NDT * B], F16, kind="ExternalInput")
    wqp = nc.dram_tensor("wqp", [NDT // NWC, 128, NWC * HQ * HD], F16,
                         kind="ExternalInput")
    wkp = nc.dram_tensor("wkp", [128, NDT * HD], F16, kind="ExternalInput")
    wvp = nc.dram_tensor("wvp", [128, NDT * HD], F16, kind="ExternalInput")
    wop = nc.dram_tensor("wop", [128, HQ * DIM], F16, kind="ExternalInput")
    kT = nc.dram_tensor("kT", [B, HD, T], F16, kind="ExternalInput")
    v = nc.dram_tensor("v", [B, 128, T], VDT, kind="ExternalInput")
    out = nc.dram_tensor("out", [B, DIM], F32, kind="ExternalOutput")

    with tile.TileContext(nc) as tc:
        _emit(nc, tc, xs, wqp, wkp, wvp, wop, kT, v, out)
    nc.compile()
    return nc


def _emit(nc, tc, xs, wqp, wkp, wvp, wop, kT, v, out):
    from contextlib import ExitStack

    with ExitStack() as ctx:
        const = ctx.enter_context(tc.tile_pool(name="const", bufs=1))
        wpool = ctx.enter_context(tc.tile_pool(name="weights", bufs=3))
        wopool = ctx.enter_context(tc.tile_pool(name="wopool", bufs=2))
        kpool = ctx.enter_context(tc.tile_pool(name="kpool", bufs=4))
        vpool = ctx.enter_context(tc.tile_pool(name="vpool", bufs=4))

        ident = const.tile([16, 16], F32, tag="ident")

        # x^T in f16: [128, (dt, b)] — single contiguous load
        xs_h = const.tile([128, NDT * B], F16, tag="xs_h")
        nc.sync.dma_start(xs_h[:], xs[:])

        QT = const.tile([128, HQ * B], F32, tag="QT")       # [d, (h,b)] fp32
        QTh = const.tile([128, HQ * B], F16, tag="QTh")     # fp16 copy
        KTnh = const.tile([128, B], F16, tag="KTnh")        # new-token K^T f16
        vrowh = const.tile([1, B * HD], F16, tag="vrowh")   # new-token V f16
        AT = const.tile([128, HQ * B], F16, tag="AT")       # attn out^T f16
        wo_h = const.tile([128, HQ * DIM], F16, tag="wo_h")  # resident f16 wo
        wk_h = const.tile([128, NDT * HD], F16, tag="wk_h")
        wv_h = const.tile([128, NDT * HD], F16, tag="wv_h")
        q_s = const.tile([B, HQ * HD], F32, tag="q_s")
        kn_s = const.tile([B, HD], F32, tag="kn_s")
        vn_s = const.tile([B, HD], F32, tag="vn_s")
        ones_h = const.tile([128, 1], F16, tag="ones_h")
        ones_r = const.tile([1, 128], F16, tag="ones_r")
        rcall = const.tile([1, HQ * B], F16, tag="rcall")   # 1/den, (h b)

        make_identity(nc, ident[:])
        nc.vector.memset(ones_h[:], 1.0)
        nc.vector.memset(ones_r[:], 1.0)

        kt_t, vb_t, pt_t = {}, {}, {}

        def load(b):
            # Contiguous loads, consumption order, single HWDGE ring.
            # (K columns pre-permuted on host to the t = 64*p + n order
            # that V's layout produces.)
            ktb = kpool.tile([128, T], F16, tag="ktb")
            nc.sync.dma_start(ktb[:], kT[b])
            kt_t[b] = ktb
            vb = vpool.tile([128, T], VDT, tag="vb")
            nc.sync.dma_start(vb[:], v[b])
            vb_t[b] = vb

        # ---------------- phase 0: projections (f16 PE) ----------------
        with tc.tile_pool(name="psum0", bufs=1, space="PSUM") as pp0:
            qp = pp0.tile([B, HQ * HD], F32, tag="qp")
            knp = pp0.tile([B, HD], F32, tag="knp")
            vnp = pp0.tile([B, HD], F32, tag="vnp")

            for c in range(NDT // NWC):
                wq_h = wpool.tile([128, NWC * HQ * HD], F16, tag="wq_h")
                nc.sync.dma_start(wq_h[:], wqp[c])
                for t in range(NWC):
                    dt = c * NWC + t
                    nc.tensor.matmul(
                        qp[:], xs_h[:, dt * B:(dt + 1) * B],
                        wq_h[:, t * HQ * HD:(t + 1) * HQ * HD],
                        start=(dt == 0), stop=(dt == NDT - 1),
                    )

            # DMA ring: K0/V0 ahead of wk/wv so scores(0) starts earliest
            load(0)
            nc.sync.dma_start(wk_h[:], wkp[:])
            nc.sync.dma_start(wv_h[:], wvp[:])
            load(1)
            load(2)
            load(3)

            for dt in range(NDT):
                nc.tensor.matmul(
                    knp[:], xs_h[:, dt * B:(dt + 1) * B],
                    wk_h[:, dt * HD:(dt + 1) * HD],
                    start=(dt == 0), stop=(dt == NDT - 1),
                )
            for dt in range(NDT):
                nc.tensor.matmul(
                    vnp[:], xs_h[:, dt * B:(dt + 1) * B],
                    wv_h[:, dt * HD:(dt + 1) * HD],
                    start=(dt == 0), stop=(dt == NDT - 1),
                )

            nc.vector.tensor_copy(q_s[:], qp[:])
            nc.vector.tensor_copy(kn_s[:], knp[:])
            nc.vector.tensor_copy(vn_s[:], vnp[:])

            # v_new rows (f16) flattened onto partition 0 (SWDGE cast DMA)
            nc.gpsimd.dma_start(
                vrowh[:].rearrange("p (b c) -> p b c", c=HD)[0:1, :, :],
                vn_s[:],
            )

            # transposes: q [16,512] -> QT [128, (h,b)]; k_new -> KTn (f16)
            for h in range(HQ):
                tp = pp0.tile([128, B], F32, tag="tp", bufs=2)
                nc.tensor.transpose(
                    tp[:], q_s[:, h * HD:(h + 1) * HD], ident[:]
                )
                nc.vector.tensor_copy(QT[:, h * B:(h + 1) * B], tp[:])
            tpk = pp0.tile([128, B], F32, tag="tp", bufs=2)
            nc.tensor.transpose(tpk[:], kn_s[:], ident[:])
            nc.vector.tensor_copy(KTnh[:], tpk[:])
            nc.vector.tensor_copy(QTh[:], QT[:])

        # ---------------- phase 1: attention over the cache ----------------
        # Software-pipelined: burst b = [scores(b) | pv(b-1)] has no
        # PE-blocking dependencies; DVE/ACT work runs concurrently.
        QTh3 = QTh[:].rearrange("p (h b) -> p b h", b=B)   # [128, b, 4]
        vrowh3 = vrowh[:].rearrange("p (b c) -> p b c", c=HD)
        AT3 = AT[:].rearrange("p (h b) -> p b h", b=B)
        rcall3 = rcall[:].rearrange("p (h b) -> p b h", b=B)

        with (
            tc.tile_pool(name="ptpool", bufs=2) as ptpool,
            tc.tile_pool(name="small", bufs=3) as small,
            tc.tile_pool(name="stpsum", bufs=2, space="PSUM") as stpsum,
            tc.tile_pool(name="opsum", bufs=2, space="PSUM") as opsum,
            tc.tile_pool(name="denpsum", bufs=2, space="PSUM") as denpsum,
        ):
            def scores(b):
                # scores^T tiles: [t'(128), h(4)] per cache tile + new token
                stp = stpsum.tile([128, SW], F32, tag="stp")
                qb = QTh3[:, b, :]
                ktb = kt_t.pop(b)
                for n in range(NT):
                    nc.tensor.matmul(
                        stp[:, 4 * n:4 * n + 4],
                        ktb[:, 128 * n:128 * (n + 1)],
                        qb,
                        start=True, stop=True,
                    )
                nc.tensor.matmul(
                    stp[0:1, 4 * NT:SW], KTnh[:, b:b + 1], qb,
                    start=True, stop=True,
                )
                pt = ptpool.tile([128, SW], F16, tag="pt")
                nc.scalar.activation(pt[:, 0:4 * NT], stp[:, 0:4 * NT], Exp,
                                     scale=SCALE)
                nc.scalar.activation(
                    pt[0:1, 4 * NT:SW], stp[0:1, 4 * NT:SW], Exp, scale=SCALE,
                )
                pt_t[b] = pt

            def pv(b):
                # op [d(128), h(4)] += V-tile.T @ P-tile: V is the stationary
                # operand (128-col fp8 -> FWL), P streams 4 columns.  The
                # output is already [d, h] -- no transpose needed.
                pt = pt_t.pop(b)
                vb = vb_t.pop(b)
                op = opsum.tile([HD, HQ], F32, tag="op")
                for n in range(NT):
                    nc.tensor.matmul(
                        op[:],
                        vb[:, 128 * n:128 * (n + 1)],
                        pt[:, 4 * n:4 * n + 4],
                        start=(n == 0), stop=False,
                    )
                nc.tensor.matmul(
                    op[:], vrowh3[0:1, b, :], pt[0:1, 4 * NT:SW],
                    start=False, stop=True,
                )

                # softmax denominators: ones.T @ P -> [1, (g h)]
                dps = denpsum.tile([1, SW], F32, tag="dps")
                nc.tensor.matmul(
                    dps[0:1, 0:4 * NT], ones_h[:], pt[:, 0:4 * NT],
                    start=True, stop=True,
                )
                nc.tensor.matmul(
                    dps[0:1, 4 * NT:SW], ones_h[0:1, 0:1], pt[0:1, 4 * NT:SW],
                    start=True, stop=True,
                )
                # async DVE: reduce + reciprocal into rcall, AT copy
                dred = small.tile([1, HQ], F32, tag="dred")
                nc.vector.reduce_sum(
                    dred[:].rearrange("p h -> p h ()"),
                    dps[:].rearrange("p (g h) -> p h g", h=HQ),
                    axis=mybir.AxisListType.X,
                )
                with nc.allow_low_precision(reason="f16 1/den scale factors"):
                    nc.vector.reciprocal(rcall3[:, b, :], dred[:])
                nc.vector.tensor_copy(AT3[:, b, :], op[:])

            for b in range(B):
                scores(b)
                if b >= 1:
                    pv(b - 1)
                if b + 4 < B:
                    load(b + 4)
                if b == 8:
                    # wo f16, one contiguous load queued mid-ring so it is
                    # resident well before phase 2
                    nc.sync.dma_start(wo_h[:], wop[:])
            pv(B - 1)

        # ---------------- phase 2: output projection (f16, resident wo) ----
        with (
            tc.tile_pool(name="wopsum", bufs=2, space="PSUM") as wopsum,
        ):
            # broadcast 1/den [1, (h b)] to all 128 partitions via a
            # ones-matmul, then rescale AT in one DVE op
            rcb = wopsum.tile([128, HQ * B], F32, tag="rcb")
            nc.tensor.matmul(rcb[:], ones_r[:], rcall[:], start=True, stop=True)
            with nc.allow_low_precision(reason="f16 attn rescale"):
                nc.vector.tensor_tensor(AT[:], AT[:], rcb[:], Mult)
            for q in range(4):                      # 1024-col output blocks
                wop_t = wopsum.tile([B, 1024], F32, tag="wop")
                for c in range(HQ):
                    for ns in range(2):
                        nc.tensor.matmul(
                            wop_t[:, 512 * ns:512 * (ns + 1)],
                            AT[:, B * c:B * (c + 1)],
                            wo_h[:, c * DIM + 1024 * q + 512 * ns:
                                 c * DIM + 1024 * q + 512 * (ns + 1)],
                            start=(c == 0), stop=(c == HQ - 1),
                        )
                wos = wopool.tile([B, 1024], F32, tag="wos")
                nc.vector.tensor_copy(wos[:], wop_t[:])
                nc.sync.dma_start(out[:, 1024 * q:1024 * (q + 1)], wos[:])

_NC = None


def _get_nc():
    global _NC
    if _NC is None:
        _NC = _build_nc()
    return _NC


def make_in_maps(inputs):
    x = np.asarray(inputs["x"], dtype=np.float32)
    ck = np.asarray(inputs["cache_k"], dtype=np.float32)
    cv = np.asarray(inputs["cache_v"], dtype=np.float32)
    wq = np.asarray(inputs["wq"], dtype=np.float32)
    wk = np.asarray(inputs["wk"], dtype=np.float32)
    wv = np.asarray(inputs["wv"], dtype=np.float32)
    wo = np.asarray(inputs["wo"], dtype=np.float32)

    # xs: x^T [DIM, B] -> [128, (dt b)] f16
    xT = x.reshape(B, DIM).T.astype(np.float16)
    xs = np.ascontiguousarray(
        xT.reshape(NDT, 128, B).transpose(1, 0, 2).reshape(128, NDT * B)
    )
    wqT = wq.T.astype(np.float16)    # [DIM, H*HD]
    wkT = wk.T.astype(np.float16)    # [DIM, HKV*HD]
    wvT = wv.T.astype(np.float16)

    in_maps = []
    for c in range(NCORES):
        hq0 = HQ * HD * c
        # wq slice packed to chunk layout [4][128][NWC*512]
        wqc = wqT[:, hq0:hq0 + HQ * HD]          # [4096, 512]
        wqp = np.ascontiguousarray(
            wqc.reshape(NDT // NWC, NWC, 128, HQ * HD)
            .transpose(0, 2, 1, 3)
            .reshape(NDT // NWC, 128, NWC * HQ * HD)
        )
        wkc = wkT[:, HD * c:HD * (c + 1)]        # [4096, 128]
        wkp = np.ascontiguousarray(
            wkc.reshape(NDT, 128, HD).transpose(1, 0, 2).reshape(128, NDT * HD)
        )
        wvc = wvT[:, HD * c:HD * (c + 1)]
        wvp = np.ascontiguousarray(
            wvc.reshape(NDT, 128, HD).transpose(1, 0, 2).reshape(128, NDT * HD)
        )
        # wo slice [512, 4096] -> [128, (c 4096)]
        woc = wo[:, hq0:hq0 + HQ * HD].T.astype(np.float16)   # [512, 4096]
        wop = np.ascontiguousarray(
            woc.reshape(HQ, 128, DIM).transpose(1, 0, 2).reshape(128, HQ * DIM)
        )
        # K^T with columns permuted to the t = 64*p + n interleaved order
        # (matches V's layout partition mapping).
        kTc = ck[:, :, c, :].transpose(0, 2, 1).astype(np.float16)  # [B,128d,8192t]
        kTc = np.ascontiguousarray(
            kTc.reshape(B, HD, 128, NT).transpose(0, 1, 3, 2).reshape(B, HD, T)
        )
        # V natural layout [B, T, HD] == [B, 128, (n d)] with t = 64p + n
        vc = np.ascontiguousarray(cv[:, :, c, :]).astype(VDT_NP).reshape(B, 128, T)
        in_maps.append({
            "xs": xs,
            "wqp": wqp,
            "wkp": wkp,
            "wvp": wvp,
            "wop": wop,
            "kT": kTc,
            "v": vc,
        })
    return in_maps


def run(in_maps, trace=False):
    nc = _get_nc()
    return run_bass_kernel_spmd(nc, in_maps, list(range(NCORES)), trace=trace)


def kernel(**inputs):
    res = run(make_in_maps(inputs)).results
    acc = np.zeros((B, DIM), dtype=np.float64)
    for r in res:
        acc += r["out"]
    return acc.astype(np.float32).reshape(B, 1, DIM)
